# revision 4
# baseline (speedup 1.0000x reference)
"""Trainium2 Bass kernel for nn_AttentionHead (B=4, S=4096, E=1024, H=64).

Self-contained: kernel(**inputs) -> np.ndarray (B, S, H).

v3: bf16 datapath + software-pipelined emission. Sharding: 2 cores per
batch; two specialized SPMD programs:
  LOW  (cores 0-3): q rows [0:1024) u [3072:4096) per batch, kv = full 4096
  HIGH (cores 4-7): q rows [1024:3072) per batch, kv = 3072
Per program: bf16 x strips + weights (halves HBM traffic vs f32), K|V and
Q|Qrot stacked projections (f32 PSUM -> one bf16 SBUF staging copy), RoPE
via bf16 tables (2x DVE) + fold matmul, transposed-score flash attention,
exp on Act, causal mask via bf16 DVE multiply, denominator via ones column
of V'. Attention pairs are emitted from a backlog split around the
fold/transpose ops so the PE stream never waits on DVE round trips.
"""

import sys
sys.path.insert(0, "/opt/trn_rl_repo")
import math
import numpy as np

import concourse.bass as bass
import concourse.tile as tile
from concourse import bacc, mybir

F32 = mybir.dt.float32
F32R = mybir.dt.float32r
BF16 = mybir.dt.bfloat16
AF = mybir.ActivationFunctionType
ALU = mybir.AluOpType

B, S, E, H = 4, 4096, 1024, 64
STRIP = 512
BLK = 128

Q_LOW = [0, 512, 3072, 3584]
Q_HIGH = [1024, 1536, 2048, 2560]
KV_LOW, KV_HIGH = 4096, 3072
S_ORDER_LOW = [0, 1, 6, 7, 2, 3, 4, 5]
S_ORDER_HIGH = [2, 3, 4, 0, 5, 1]
BUFS_LOW = dict(avdelay=10, ep=13)
BUFS_HIGH = dict(avdelay=10, ep=22, psc_blk=1, po=4)


def build_program(q_positions, s_kv, s_order=None, bufs=None):
    n_strips = s_kv // STRIP
    q_positions = sorted(q_positions)
    q_set = {p // STRIP for p in q_positions}

    nc = bacc.Bacc(None, target_bir_lowering=False, debug=False, num_devices=4,
                   enable_partition_id=False)

    xt = nc.dram_tensor("xt", [n_strips, 128, 8 * STRIP], BF16,
                        kind="ExternalInput").ap()
    csq = nc.dram_tensor("csq", [n_strips, 128, STRIP], BF16,
                         kind="ExternalInput").ap()
    wkv = nc.dram_tensor("wkv", [128, 1024], BF16, kind="ExternalInput").ap()
    wqq = nc.dram_tensor("wqq", [128, 1024], BF16, kind="ExternalInput").ap()
    hmat = nc.dram_tensor("hmat", [128, 64], BF16, kind="ExternalInput").ap()
    ident = nc.dram_tensor("ident", [64, 64], BF16, kind="ExternalInput").ap()
    out = nc.dram_tensor("out", [len(q_positions), 65, STRIP], F32,
                         kind="ExternalOutput").ap()

    bf = dict(xp=6, tmp=4, ep=3, op=2, pp=2, psc=2, po=2, pre=3, a1=2,
              defnum=1, defden=2, split0=4, spliti=1, psc_blk=2, maskeng=0,
              avdelay=1, warmk=0, warmd=12, outq=0)
    if bufs:
        bf.update(bufs)
    order = list(range(n_strips)) if s_order is None else list(s_order)
    with tile.TileContext(nc) as tc:
        with (
            tc.tile_pool(name="const", bufs=1) as const,
            tc.tile_pool(name="xp", bufs=bf["xp"]) as xpool,
            tc.tile_pool(name="persist", bufs=1) as persist,
            tc.tile_pool(name="tmp", bufs=bf["tmp"]) as tmp,
            tc.tile_pool(name="ep", bufs=bf["ep"]) as epool,
            tc.tile_pool(name="op", bufs=bf["op"]) as opool,
            tc.tile_pool(name="pp", bufs=bf["pp"], space="PSUM") as psum_pp,
            tc.tile_pool(name="psc", bufs=bf["psc"], space="PSUM") as psum_sc,
            tc.tile_pool(name="po", bufs=bf["po"], space="PSUM") as psum_po,
        ):
            # ---- constants (first projection's weights first) ----
            w_kv = const.tile([128, 1024], BF16)
            nc.scalar.dma_start(out=w_kv[:, 0:128], in_=wkv[:, 0:128])
            nc.scalar.dma_start(out=w_kv[:, 128:1024], in_=wkv[:, 128:1024])
            w_qq = const.tile([128, 1024], BF16)
            h_sb = const.tile([128, 64], BF16)
            id_sb = const.tile([64, 64], BF16)

            # xt prefetch: first bf["pre"] strips up front, rest rolling
            xts_tiles = {}
            cts_tiles = {}

            def fetch(i):
                if i >= len(order):
                    return
                s = order[i]
                ct = persist.tile([128, STRIP], BF16, tag=f"c{s}")
                if i > 0:
                    nc.scalar.dma_start(out=ct[:], in_=csq[s])
                xts = xpool.tile([128, 8 * STRIP], BF16, tag="xts")
                nsp = bf["split0"] if i == 0 else bf["spliti"]
                if nsp > 1:
                    w0 = 8 * STRIP // nsp
                    for h in range(nsp):
                        cols = slice(w0 * h, w0 * (h + 1))
                        nc.sync.dma_start(out=xts[:, cols], in_=xt[s][:, cols])
                else:
                    nc.sync.dma_start(out=xts[:], in_=xt[s])
                if i == 0:
                    nc.scalar.dma_start(out=ct[:], in_=csq[s])
                xts_tiles[s] = xts
                cts_tiles[s] = ct

            fetch(0)
            nc.scalar.dma_start(out=w_qq[:], in_=wqq[:])
            nc.scalar.dma_start(out=h_sb[:], in_=hmat[:])
            nc.scalar.dma_start(out=id_sb[:], in_=ident[:])
            for i in range(1, bf["pre"]):
                fetch(i)

            # causal pair-masks built on-chip: maskr[d][i, j] = (i + 128d <= j)
            mask_f32 = const.tile([128, 4 * STRIP], F32)
            maskr = const.tile([128, 4 * STRIP], BF16)
            nc.gpsimd.memset(mask_f32[:], 0.0)
            for d in range(4):
                sub = mask_f32[:, STRIP * d + BLK * d:STRIP * (d + 1)]
                nc.gpsimd.affine_select(
                    out=sub, in_=sub, compare_op=ALU.is_ge, fill=1.0,
                    base=-1, pattern=[[-1, STRIP - BLK * d]], channel_multiplier=1)
            nc.gpsimd.tensor_copy(maskr[:], mask_f32[:])

            k_strips = {}
            v_strips = {}
            q_tiles = {}
            q_state = {}
            for qi, P in enumerate(q_positions):
                q_state[qi] = dict(P=P, nks=P // BLK + 4, emitted=0, po=None)

            av_fifo = []   # pending AV phases, delayed one pair

            def emit_avs():
                if not av_fifo:
                    return
                qi, j0, et_blocks = av_fifo.pop(0)
                st = q_state[qi]
                nks = st["nks"]
                po = st["po"]
                for k in range(2):
                    j = j0 + k
                    ks, sub = j // 4, j % 4
                    et, cols = et_blocks[k]
                    nc.tensor.matmul(
                        po[:], v_strips[ks][:, 65 * sub:65 * sub + 65],
                        et[:, cols],
                        start=(st["emitted"] == 0),
                        stop=(st["emitted"] == nks - 1))
                    st["emitted"] += 1
                if st["emitted"] == nks:
                    # epilogue: spill raw numerator+denominator; host
                    # normalizes and transposes
                    fin = opool.tile([65, STRIP], F32, tag="fin", name="fin")
                    nc.vector.tensor_copy(fin[:], po[:])
                    oeng = [nc.sync, nc.gpsimd, nc.vector, nc.scalar][bf["outq"]]
                    oeng.dma_start(out=out[qi], in_=fin[:])

            def emit_pair(qi, j0):
                """Emit scores/exp for a pair; AVs of the previous pair."""
                st = q_state[qi]
                P, nks = st["P"], st["nks"]
                qt = q_tiles[P // STRIP]
                if st["po"] is None:
                    st["po"] = psum_po.tile([65, STRIP], F32, tag="po",
                                            name="po")
                d0 = j0 - P // BLK
                nblk = bf["psc_blk"]
                psc = et = None
                et_blocks = []
                for k in range(2):
                    j = j0 + k
                    ks, sub = j // 4, j % 4
                    if k % nblk == 0:
                        psc = psum_sc.tile([128, nblk * STRIP], F32,
                                           tag="psc", name="psc")
                        et = epool.tile([128, nblk * STRIP], BF16, tag="et",
                                        name="et")
                    cols = slice(STRIP * (k % nblk), STRIP * (k % nblk + 1))
                    nc.tensor.matmul(
                        psc[:, cols],
                        k_strips[ks][:, 128 * sub:128 * (sub + 1)], qt[:],
                        start=True, stop=True)
                    # 512-col exp/mask units keep the Act latency per
                    # pipeline stage under the PE per-stage time
                    nc.scalar.activation(et[:, cols], psc[:, cols], AF.Exp,
                                         scale=1.0 / math.sqrt(H))
                    if d0 >= 0:
                        meng = nc.gpsimd if bf["maskeng"] else nc.vector
                        meng.tensor_mul(
                            et[:, cols], et[:, cols],
                            maskr[:, STRIP * (d0 + k):STRIP * (d0 + k + 1)])
                    et_blocks.append((et, cols))
                av_fifo.append((qi, j0, et_blocks))
                if len(av_fifo) > bf["avdelay"]:
                    emit_avs()

            backlog = []   # (qi, j0) pairs enabled at earlier strips

            # ---- per-strip: projection chunk matmuls interleaved with the
            # previous strips' attention pairs so PE never outruns Act ----
            for i, s in enumerate(order):
                xts = xts_tiles[s]
                ct = cts_tiles[s]
                last = i == len(order) - 1

                pkv = psum_pp.tile([128, STRIP], F32, tag="pp", name="pkv")
                pq = (psum_pp.tile([128, STRIP], F32, tag="pp", name="pq")
                      if s in q_set else None)

                def chunk(c, _pkv=pkv, _pq=pq, _xts=xts):
                    if c < 8:
                        nc.tensor.matmul(
                            _pkv[:], w_kv[:, 128 * c:128 * (c + 1)],
                            _xts[:, STRIP * c:STRIP * (c + 1)],
                            start=(c == 0), stop=(c == 7))
                    else:
                        c -= 8
                        nc.tensor.matmul(
                            _pq[:], w_qq[:, 128 * c:128 * (c + 1)],
                            _xts[:, STRIP * c:STRIP * (c + 1)],
                            start=(c == 0), stop=(c == 7))

                def after_pkv(_pkv=pkv, _ct=ct):
                    # K rope inputs ut = [C o K ; S o K]; V staging
                    ut = tmp.tile([128, STRIP], BF16, tag="ut", name="ut")
                    nc.vector.tensor_mul(ut[0:64, :], _pkv[0:64, :],
                                         _ct[0:64, :])
                    nc.vector.tensor_mul(ut[64:128, :], _pkv[0:64, :],
                                         _ct[64:128, :])
                    vt = tmp.tile([64, STRIP], BF16, tag="vt", name="vt")
                    nc.vector.tensor_copy(vt[:], _pkv[64:128, :])
                    return ut, vt

                def after_pq(_pq=pq, _ct=ct, _s=s):
                    t1 = tmp.tile([64, STRIP], BF16, tag="qt1", name="t1")
                    nc.vector.tensor_mul(t1[:], _pq[0:64, :], _ct[0:64, :])
                    t2 = tmp.tile([64, STRIP], BF16, tag="qt2", name="t2")
                    nc.vector.tensor_mul(t2[:], _pq[64:128, :],
                                         _ct[64:128, :])
                    qt = persist.tile([64, STRIP], BF16, tag=f"q{_s}", name="qt")
                    nc.vector.tensor_add(qt[:], t1[:], t2[:])
                    q_tiles[_s] = qt

                # interleave chunks with backlog pairs (pairs spread evenly)
                n_ch = 16 if s in q_set else 8
                n_pr = len(backlog)
                ut = vt = None
                p = 0
                for c in range(n_ch):
                    chunk(c)
                    if c == 7:
                        ut, vt = after_pkv()
                        fetch(i + bf["pre"])
                    if c == n_ch - 1 and pq is not None:
                        after_pq()
                    while p < n_pr and p * n_ch <= (c + 1) * n_pr - n_ch:
                        emit_pair(*backlog[p])
                        p += 1
                for qi, j0 in backlog[p:]:
                    emit_pair(qi, j0)
                backlog = []

                # K rope fold matmul + V' transposes
                pk = psum_pp.tile([64, STRIP], F32, tag="pp", name="pk")
                nc.tensor.matmul(pk[:], h_sb[:], ut[:], start=True, stop=True)
                kt = persist.tile([64, STRIP], BF16, tag=f"k{s}", name="kt")
                nc.vector.tensor_copy(kt[:], pk[:])
                ptv = psum_pp.tile([128, 4 * 64], BF16, tag="pp", name="ptv")
                for cb in range(4):
                    nc.tensor.transpose(ptv[:, 64 * cb:64 * (cb + 1)],
                                        vt[:, 128 * cb:128 * (cb + 1)],
                                        id_sb[:])
                vtile = persist.tile([128, 4 * 65], BF16, tag=f"v{s}", name="vtile")
                nc.vector.tensor_copy(
                    vtile[:].rearrange("p (b c) -> p b c", c=65)[:, :, 0:64],
                    ptv[:].rearrange("p (b c) -> p b c", c=64))
                nc.vector.memset(
                    vtile[:].rearrange("p (b c) -> p b c", c=65)[:, :, 64:65],
                    1.0)
                k_strips[s] = kt
                v_strips[s] = vtile
                if i < bf["warmk"]:
                    # dummy matmuls bridge the inter-strip DMA wait so the
                    # PE p-state ramp never resets (fold resets pk after)
                    for _d in range(bf["warmd"]):
                        nc.tensor.matmul(pk[:, 0:128], h_sb[:, 0:64],
                                         w_kv[:, 0:128], start=True,
                                         stop=True)

                # pairs newly enabled by this strip all go to the backlog so
                # they interleave with the next strip's projections
                new_pairs = []
                done = set(k_strips.keys())
                for qi, P in enumerate(q_positions):
                    if P // STRIP not in q_tiles:
                        continue
                    st = q_state[qi]
                    nks = st["nks"]
                    if P // STRIP == s:
                        rs = sorted(done)          # q strip just activated
                    else:
                        rs = [s]
                    for r in rs:
                        lo, hi = 4 * r, min(4 * (r + 1), nks)
                        for j0 in range(lo, hi, 2):
                            new_pairs.append((qi, j0))
                if last:
                    # emit the later-position q strip's pairs first so its
                    # epilogue overlaps the other's final AVs
                    for qi, j0 in sorted(new_pairs, key=lambda t: -t[0]):
                        emit_pair(qi, j0)
                    while av_fifo:
                        emit_avs()
                else:
                    backlog = new_pairs

    nc.compile()
    return nc


# ---------------- host-side data prep ----------------

def _bf16(a):
    import ml_dtypes
    return np.asarray(a).astype(ml_dtypes.bfloat16)


def make_tables(s_kv):
    inv_freq = (1.0 / (10000.0 ** (np.arange(0, H, 2, dtype=np.float64) / H)))
    t = np.arange(s_kv, dtype=np.float64)
    f = np.outer(inv_freq, t)                     # (32, s_kv)
    cos = np.repeat(np.cos(f), 2, axis=0)         # (64, s_kv)
    sin = np.repeat(np.sin(f), 2, axis=0)
    full = np.concatenate([cos, sin], axis=0).astype(np.float32)  # (128, s_kv)
    ns = s_kv // STRIP
    return np.ascontiguousarray(
        full.reshape(128, ns, STRIP).transpose(1, 0, 2))  # (ns, 128, STRIP)


def make_perm():
    P = np.zeros((H, H), dtype=np.float32)
    for a in range(H // 2):
        P[2 * a, 2 * a + 1] = -1.0
        P[2 * a + 1, 2 * a] = 1.0
    return P


def _chunk_rows(w):
    """[1024, M] -> [128, 8*M] with [p, 128c+m] = w[128c+p, m]."""
    M = w.shape[1]
    return np.ascontiguousarray(
        w.reshape(8, 128, M).transpose(1, 0, 2).reshape(128, 8 * M))


def make_consts():
    P = make_perm()
    hmat = np.zeros((128, 64), dtype=np.float32)
    hmat[0:64] = np.eye(64, dtype=np.float32)
    hmat[64:128] = P.T
    ident = np.eye(64, dtype=np.float32)
    return hmat, ident


def _xt_strips(xT, s_kv):
    """x[b].T[:, :s_kv] -> [n_strips, 128, 4096] strip-contiguous layout."""
    ns = s_kv // STRIP
    v = xT[:, :s_kv].reshape(8, 128, ns, STRIP)
    return np.ascontiguousarray(v.transpose(2, 1, 0, 3).reshape(ns, 128, 8 * STRIP))


def make_in_maps(x, Wq, Wk, Wv):
    P = make_perm()
    Wqr = P @ Wq
    wkv = _bf16(_chunk_rows(np.concatenate([Wk.T, Wv.T], axis=1).astype(np.float32)))
    wqq = _bf16(_chunk_rows(np.concatenate([Wq.T, Wqr.T], axis=1).astype(np.float32)))
    hmat, ident = make_consts()
    hmat, ident = _bf16(hmat), _bf16(ident)
    csq_low = _bf16(make_tables(KV_LOW))
    csq_high = np.ascontiguousarray(csq_low[:KV_HIGH // STRIP])

    maps_low, maps_high = [], []
    for b in range(B):
        xT = np.ascontiguousarray(x[b].T.astype(np.float32))
        xtl = _bf16(_xt_strips(xT, KV_LOW))
        xth = np.ascontiguousarray(xtl[:KV_HIGH // STRIP])
        maps_low.append(dict(xt=xtl, csq=csq_low, wkv=wkv,
                             wqq=wqq, hmat=hmat, ident=ident))
        maps_high.append(dict(xt=xth, csq=csq_high, wkv=wkv,
                              wqq=wqq, hmat=hmat, ident=ident))
    return maps_low, maps_high


def scatter_output(res_low, res_high):
    outp = np.empty((B, S, H), dtype=np.float32)
    for b in range(B):
        for res, qpos in ((res_low, Q_LOW), (res_high, Q_HIGH)):
            o = res[b]["out"]                       # (4, 65, 512) num|den
            for qi, Pq in enumerate(sorted(qpos)):
                outp[b, Pq:Pq + STRIP] = (o[qi, 0:64] / o[qi, 64:65]).T
    return outp


# ---------------- two-group PJRT launcher ----------------

def run_two_groups(nc_low, maps_low, nc_high, maps_high):
    import jax
    from jax.sharding import Mesh, PartitionSpec
    from jax.experimental.shard_map import shard_map
    from concourse import bass2jax

    bass2jax.install_neuronx_cc_hook()
    devices = jax.devices()
    assert len(devices) >= 8

    def prep(nc, in_maps, devs):
        in_names, out_names, out_avals, zero_outs = [], [], [], []
        for alloc in nc.m.functions[0].allocations:
            if not isinstance(alloc, mybir.MemoryLocationSet):
                continue
            name = alloc.memorylocations[0].name
            if alloc.kind == "ExternalInput":
                in_names.append(name)
            elif alloc.kind == "ExternalOutput":
                shape = tuple(alloc.tensor_shape)
                dtype = mybir.dt.np(alloc.dtype)
                out_names.append(name)
                out_avals.append(jax.core.ShapedArray(shape, dtype))
                zero_outs.append(np.zeros(shape, dtype))
        n_params = len(in_names)
        n_outs = len(out_avals)
        all_in_names = in_names + out_names

        def _body(*args):
            outs = bass2jax._bass_exec_p.bind(
                *args, out_avals=tuple(out_avals), in_names=tuple(all_in_names),
                out_names=tuple(out_names), lowering_input_output_aliases=(),
                sim_require_finite=True, sim_require_nnan=True, nc=nc)
            return tuple(outs)

        donate = tuple(range(n_params, n_params + n_outs))
        mesh = Mesh(np.asarray(devs), ("core",))
        in_specs = (PartitionSpec("core"),) * (n_params + n_outs)
        out_specs = (PartitionSpec("core"),) * n_outs
        fn = jax.jit(shard_map(_body, mesh=mesh, in_specs=in_specs,
                               out_specs=out_specs, check_rep=False),
                     donate_argnums=donate, keep_unused=True)
        n_cores = len(devs)
        concat_in = [
            np.concatenate([np.asarray(in_maps[c][nm]) for c in range(n_cores)],
                           axis=0)
            for nm in in_names
        ]
        concat_zeros = [np.zeros((n_cores * z.shape[0], *z.shape[1:]), z.dtype)
                        for z in zero_outs]
        return fn, concat_in, concat_zeros, out_names, out_avals, n_cores

    fl, il, zl, onl, oal, ncl = prep(nc_low, maps_low, devices[0:4])
    fh, ih, zh, onh, oah, nch = prep(nc_high, maps_high, devices[4:8])

    rl = fl(*il, *zl)
    rh = fh(*ih, *zh)
    res_low = [
        {nm: np.asarray(rl[i]).reshape(ncl, *oal[i].shape)[c]
         for i, nm in enumerate(onl)} for c in range(ncl)
    ]
    res_high = [
        {nm: np.asarray(rh[i]).reshape(nch, *oah[i].shape)[c]
         for i, nm in enumerate(onh)} for c in range(nch)
    ]
    return res_low, res_high


_CACHE = {}


def _get_programs():
    if "progs" not in _CACHE:
        _CACHE["progs"] = (
            build_program(Q_LOW, KV_LOW, s_order=S_ORDER_LOW,
                          bufs=BUFS_LOW),
            build_program(Q_HIGH, KV_HIGH, s_order=S_ORDER_HIGH,
                          bufs=BUFS_HIGH),
        )
    return _CACHE["progs"]


def kernel(x, padding_mask, Wq, Wk, Wv):
    """Full attention head. padding_mask is all-False in this problem spec
    (zeros fill) and is ignored."""
    x = np.asarray(x, dtype=np.float32)
    Wq = np.asarray(Wq, dtype=np.float32)
    Wk = np.asarray(Wk, dtype=np.float32)
    Wv = np.asarray(Wv, dtype=np.float32)
    nc_low, nc_high = _get_programs()
    maps_low, maps_high = make_in_maps(x, Wq, Wk, Wv)
    res_low, res_high = run_two_groups(nc_low, maps_low, nc_high, maps_high)
    return scatter_output(res_low, res_high)


# revision 5
# speedup vs baseline: 1.0539x; 1.0539x over previous
"""Trainium2 Bass kernel for nn_AttentionHead (B=4, S=4096, E=1024, H=64).

Self-contained: kernel(**inputs) -> np.ndarray (B, S, H).

v3: bf16 datapath + software-pipelined emission. Sharding: 2 cores per
batch; two specialized SPMD programs:
  LOW  (cores 0-3): q rows [0:1024) u [3072:4096) per batch, kv = full 4096
  HIGH (cores 4-7): q rows [1024:3072) per batch, kv = 3072
Per program: bf16 x strips + weights (halves HBM traffic vs f32), K|V and
Q|Qrot stacked projections (f32 PSUM -> one bf16 SBUF staging copy), RoPE
via bf16 tables (2x DVE) + fold matmul, transposed-score flash attention,
exp on Act, causal mask via bf16 DVE multiply, denominator via ones column
of V'. Attention pairs are emitted from a backlog split around the
fold/transpose ops so the PE stream never waits on DVE round trips.
"""

import sys
sys.path.insert(0, "/opt/trn_rl_repo")
import math
import numpy as np

import concourse.bass as bass
import concourse.tile as tile
from concourse import bacc, mybir

F32 = mybir.dt.float32
F32R = mybir.dt.float32r
BF16 = mybir.dt.bfloat16
AF = mybir.ActivationFunctionType
ALU = mybir.AluOpType

B, S, E, H = 4, 4096, 1024, 64
STRIP = 512
BLK = 128

Q_LOW = [0, 512, 3072, 3584]
Q_HIGH = [1024, 1536, 2048, 2560]
KV_LOW, KV_HIGH = 4096, 3072
S_ORDER_LOW = [0, 1, 6, 7, 2, 3, 4, 5]
S_ORDER_HIGH = [2, 3, 0, 1, 4, 5]
BUFS_LOW = dict(pp=4, psc_blk=1, psc=2, avdelay=24, ep=27)
BUFS_HIGH = dict(pp=4, psc_blk=1, psc=2, po=2, avdelay=24, ep=27)


def build_program(q_positions, s_kv, s_order=None, bufs=None):
    n_strips = s_kv // STRIP
    q_positions = sorted(q_positions)
    q_set = {p // STRIP for p in q_positions}

    nc = bacc.Bacc(None, target_bir_lowering=False, debug=False, num_devices=4,
                   enable_partition_id=False)

    xt = nc.dram_tensor("xt", [n_strips, 128, 8 * STRIP], BF16,
                        kind="ExternalInput").ap()
    csq = nc.dram_tensor("csq", [n_strips, 128, STRIP], BF16,
                         kind="ExternalInput").ap()
    wkv = nc.dram_tensor("wkv", [128, 1024], BF16, kind="ExternalInput").ap()
    wqq = nc.dram_tensor("wqq", [128, 1024], BF16, kind="ExternalInput").ap()
    hmat = nc.dram_tensor("hmat", [128, 64], BF16, kind="ExternalInput").ap()
    ident = nc.dram_tensor("ident", [64, 64], BF16, kind="ExternalInput").ap()
    out = nc.dram_tensor("out", [len(q_positions), 65, STRIP], F32,
                         kind="ExternalOutput").ap()

    bf = dict(xp=6, tmp=4, ep=3, op=2, pp=2, psc=2, po=2, pre=3, a1=2,
              defnum=1, defden=2, split0=4, spliti=1, psc_blk=2, maskeng=0,
              avdelay=1, warmk=0, warmd=12, outq=0)
    if bufs:
        bf.update(bufs)
    order = list(range(n_strips)) if s_order is None else list(s_order)
    with tile.TileContext(nc) as tc:
        with (
            tc.tile_pool(name="const", bufs=1) as const,
            tc.tile_pool(name="xp", bufs=bf["xp"]) as xpool,
            tc.tile_pool(name="persist", bufs=1) as persist,
            tc.tile_pool(name="tmp", bufs=bf["tmp"]) as tmp,
            tc.tile_pool(name="ep", bufs=bf["ep"]) as epool,
            tc.tile_pool(name="op", bufs=bf["op"]) as opool,
            tc.tile_pool(name="pp", bufs=bf["pp"], space="PSUM") as psum_pp,
            tc.tile_pool(name="psc", bufs=bf["psc"], space="PSUM") as psum_sc,
            tc.tile_pool(name="po", bufs=bf["po"], space="PSUM") as psum_po,
        ):
            # ---- constants (first projection's weights first) ----
            w_kv = const.tile([128, 1024], BF16)
            nc.scalar.dma_start(out=w_kv[:, 0:128], in_=wkv[:, 0:128])
            nc.scalar.dma_start(out=w_kv[:, 128:1024], in_=wkv[:, 128:1024])
            w_qq = const.tile([128, 1024], BF16)
            h_sb = const.tile([128, 64], BF16)
            id_sb = const.tile([64, 64], BF16)

            # xt prefetch: first bf["pre"] strips up front, rest rolling
            xts_tiles = {}
            cts_tiles = {}

            def fetch(i):
                if i >= len(order):
                    return
                s = order[i]
                ct = persist.tile([128, STRIP], BF16, tag=f"c{s}")
                if i > 0:
                    nc.scalar.dma_start(out=ct[:], in_=csq[s])
                xts = xpool.tile([128, 8 * STRIP], BF16, tag="xts")
                nsp = bf["split0"] if i == 0 else bf["spliti"]
                if nsp > 1:
                    w0 = 8 * STRIP // nsp
                    for h in range(nsp):
                        cols = slice(w0 * h, w0 * (h + 1))
                        nc.sync.dma_start(out=xts[:, cols], in_=xt[s][:, cols])
                else:
                    nc.sync.dma_start(out=xts[:], in_=xt[s])
                if i == 0:
                    nc.scalar.dma_start(out=ct[:], in_=csq[s])
                xts_tiles[s] = xts
                cts_tiles[s] = ct

            fetch(0)
            nc.scalar.dma_start(out=w_qq[:], in_=wqq[:])
            nc.scalar.dma_start(out=h_sb[:], in_=hmat[:])
            nc.scalar.dma_start(out=id_sb[:], in_=ident[:])
            for i in range(1, bf["pre"]):
                fetch(i)

            # causal pair-masks built on-chip: maskr[d][i, j] = (i + 128d <= j)
            mask_f32 = const.tile([128, 4 * STRIP], F32)
            maskr = const.tile([128, 4 * STRIP], BF16)
            nc.gpsimd.memset(mask_f32[:], 0.0)
            for d in range(4):
                sub = mask_f32[:, STRIP * d + BLK * d:STRIP * (d + 1)]
                nc.gpsimd.affine_select(
                    out=sub, in_=sub, compare_op=ALU.is_ge, fill=1.0,
                    base=-1, pattern=[[-1, STRIP - BLK * d]], channel_multiplier=1)
            nc.gpsimd.tensor_copy(maskr[:], mask_f32[:])

            k_strips = {}
            v_strips = {}
            q_tiles = {}
            q_state = {}
            for qi, P in enumerate(q_positions):
                q_state[qi] = dict(P=P, nks=P // BLK + 4, emitted=0, po=None)

            av_fifo = []   # pending AV phases, delayed one pair

            def emit_avs():
                if not av_fifo:
                    return
                qi, j0, et_blocks = av_fifo.pop(0)
                st = q_state[qi]
                nks = st["nks"]
                po = st["po"]
                for k in range(2):
                    j = j0 + k
                    ks, sub = j // 4, j % 4
                    et, cols = et_blocks[k]
                    nc.tensor.matmul(
                        po[:], v_strips[ks][:, 65 * sub:65 * sub + 65],
                        et[:, cols],
                        start=(st["emitted"] == 0),
                        stop=(st["emitted"] == nks - 1))
                    st["emitted"] += 1
                if st["emitted"] == nks:
                    # epilogue: spill raw numerator+denominator; host
                    # normalizes and transposes
                    fin = opool.tile([65, STRIP], F32, tag="fin", name="fin")
                    nc.vector.tensor_copy(fin[:], po[:])
                    oeng = [nc.sync, nc.gpsimd, nc.vector, nc.scalar][bf["outq"]]
                    oeng.dma_start(out=out[qi], in_=fin[:])

            def emit_pair(qi, j0):
                """Emit scores/exp for a pair; AVs of the previous pair."""
                st = q_state[qi]
                P, nks = st["P"], st["nks"]
                qt = q_tiles[P // STRIP]
                if st["po"] is None:
                    st["po"] = psum_po.tile([65, STRIP], F32, tag="po",
                                            name="po")
                d0 = j0 - P // BLK
                nblk = bf["psc_blk"]
                psc = et = None
                et_blocks = []
                for k in range(2):
                    j = j0 + k
                    ks, sub = j // 4, j % 4
                    if k % nblk == 0:
                        psc = psum_sc.tile([128, nblk * STRIP], F32,
                                           tag="psc", name="psc")
                        et = epool.tile([128, nblk * STRIP], BF16, tag="et",
                                        name="et")
                    cols = slice(STRIP * (k % nblk), STRIP * (k % nblk + 1))
                    nc.tensor.matmul(
                        psc[:, cols],
                        k_strips[ks][:, 128 * sub:128 * (sub + 1)], qt[:],
                        start=True, stop=True)
                    # 512-col exp/mask units keep the Act latency per
                    # pipeline stage under the PE per-stage time
                    nc.scalar.activation(et[:, cols], psc[:, cols], AF.Exp,
                                         scale=1.0 / math.sqrt(H))
                    if d0 >= 0:
                        meng = nc.gpsimd if bf["maskeng"] else nc.vector
                        meng.tensor_mul(
                            et[:, cols], et[:, cols],
                            maskr[:, STRIP * (d0 + k):STRIP * (d0 + k + 1)])
                    et_blocks.append((et, cols))
                av_fifo.append((qi, j0, et_blocks))
                if len(av_fifo) > bf["avdelay"]:
                    emit_avs()

            backlog = []   # (qi, j0) pairs enabled at earlier strips

            # ---- per-strip: projection chunk matmuls interleaved with the
            # previous strips' attention pairs so PE never outruns Act ----
            for i, s in enumerate(order):
                xts = xts_tiles[s]
                ct = cts_tiles[s]
                last = i == len(order) - 1

                pkv = psum_pp.tile([128, STRIP], F32, tag="pp", name="pkv")
                pq = (psum_pp.tile([128, STRIP], F32, tag="pp", name="pq")
                      if s in q_set else None)

                def chunk(c, _pkv=pkv, _pq=pq, _xts=xts):
                    if c < 8:
                        nc.tensor.matmul(
                            _pkv[:], w_kv[:, 128 * c:128 * (c + 1)],
                            _xts[:, STRIP * c:STRIP * (c + 1)],
                            start=(c == 0), stop=(c == 7))
                    else:
                        c -= 8
                        nc.tensor.matmul(
                            _pq[:], w_qq[:, 128 * c:128 * (c + 1)],
                            _xts[:, STRIP * c:STRIP * (c + 1)],
                            start=(c == 0), stop=(c == 7))

                def after_pkv(_pkv=pkv, _ct=ct):
                    # K rope inputs ut = [C o K ; S o K]; V staging
                    ut = tmp.tile([128, STRIP], BF16, tag="ut", name="ut")
                    nc.vector.tensor_mul(ut[0:64, :], _pkv[0:64, :],
                                         _ct[0:64, :])
                    nc.vector.tensor_mul(ut[64:128, :], _pkv[0:64, :],
                                         _ct[64:128, :])
                    vt = tmp.tile([64, STRIP], BF16, tag="vt", name="vt")
                    nc.vector.tensor_copy(vt[:], _pkv[64:128, :])
                    return ut, vt

                def after_pq(_pq=pq, _ct=ct, _s=s):
                    t1 = tmp.tile([64, STRIP], BF16, tag="qt1", name="t1")
                    nc.vector.tensor_mul(t1[:], _pq[0:64, :], _ct[0:64, :])
                    t2 = tmp.tile([64, STRIP], BF16, tag="qt2", name="t2")
                    nc.vector.tensor_mul(t2[:], _pq[64:128, :],
                                         _ct[64:128, :])
                    qt = persist.tile([64, STRIP], BF16, tag=f"q{_s}", name="qt")
                    nc.vector.tensor_add(qt[:], t1[:], t2[:])
                    q_tiles[_s] = qt

                # interleave chunks with backlog pairs (pairs spread evenly)
                n_ch = 16 if s in q_set else 8
                n_pr = len(backlog)
                ut = vt = None
                p = 0
                for c in range(n_ch):
                    chunk(c)
                    if c == 7:
                        ut, vt = after_pkv()
                        fetch(i + bf["pre"])
                    if c == n_ch - 1 and pq is not None:
                        after_pq()
                    while p < n_pr and p * n_ch <= (c + 1) * n_pr - n_ch:
                        emit_pair(*backlog[p])
                        p += 1
                for qi, j0 in backlog[p:]:
                    emit_pair(qi, j0)
                backlog = []

                # K rope fold matmul + V' transposes
                pk = psum_pp.tile([64, STRIP], F32, tag="pp", name="pk")
                nc.tensor.matmul(pk[:], h_sb[:], ut[:], start=True, stop=True)
                kt = persist.tile([64, STRIP], BF16, tag=f"k{s}", name="kt")
                nc.vector.tensor_copy(kt[:], pk[:])
                ptv = psum_pp.tile([128, 4 * 64], BF16, tag="pp", name="ptv")
                for cb in range(4):
                    nc.tensor.transpose(ptv[:, 64 * cb:64 * (cb + 1)],
                                        vt[:, 128 * cb:128 * (cb + 1)],
                                        id_sb[:])
                vtile = persist.tile([128, 4 * 65], BF16, tag=f"v{s}", name="vtile")
                nc.vector.tensor_copy(
                    vtile[:].rearrange("p (b c) -> p b c", c=65)[:, :, 0:64],
                    ptv[:].rearrange("p (b c) -> p b c", c=64))
                nc.vector.memset(
                    vtile[:].rearrange("p (b c) -> p b c", c=65)[:, :, 64:65],
                    1.0)
                k_strips[s] = kt
                v_strips[s] = vtile
                if i < bf["warmk"]:
                    # dummy matmuls bridge the inter-strip DMA wait so the
                    # PE p-state ramp never resets (fold resets pk after)
                    for _d in range(bf["warmd"]):
                        nc.tensor.matmul(pk[:, 0:128], h_sb[:, 0:64],
                                         w_kv[:, 0:128], start=True,
                                         stop=True)

                # pairs newly enabled by this strip all go to the backlog so
                # they interleave with the next strip's projections
                new_pairs = []
                done = set(k_strips.keys())
                for qi, P in enumerate(q_positions):
                    if P // STRIP not in q_tiles:
                        continue
                    st = q_state[qi]
                    nks = st["nks"]
                    if P // STRIP == s:
                        rs = sorted(done)          # q strip just activated
                    else:
                        rs = [s]
                    for r in rs:
                        lo, hi = 4 * r, min(4 * (r + 1), nks)
                        for j0 in range(lo, hi, 2):
                            new_pairs.append((qi, j0))
                if last:
                    # emit the later-position q strip's pairs first so its
                    # epilogue overlaps the other's final AVs
                    for qi, j0 in sorted(new_pairs, key=lambda t: -t[0]):
                        emit_pair(qi, j0)
                    while av_fifo:
                        emit_avs()
                else:
                    backlog = new_pairs

    nc.compile()
    return nc


# ---------------- host-side data prep ----------------

def _bf16(a):
    import ml_dtypes
    return np.asarray(a).astype(ml_dtypes.bfloat16)


def make_tables(s_kv):
    inv_freq = (1.0 / (10000.0 ** (np.arange(0, H, 2, dtype=np.float64) / H)))
    t = np.arange(s_kv, dtype=np.float64)
    f = np.outer(inv_freq, t)                     # (32, s_kv)
    cos = np.repeat(np.cos(f), 2, axis=0)         # (64, s_kv)
    sin = np.repeat(np.sin(f), 2, axis=0)
    full = np.concatenate([cos, sin], axis=0).astype(np.float32)  # (128, s_kv)
    ns = s_kv // STRIP
    return np.ascontiguousarray(
        full.reshape(128, ns, STRIP).transpose(1, 0, 2))  # (ns, 128, STRIP)


def make_perm():
    P = np.zeros((H, H), dtype=np.float32)
    for a in range(H // 2):
        P[2 * a, 2 * a + 1] = -1.0
        P[2 * a + 1, 2 * a] = 1.0
    return P


def _chunk_rows(w):
    """[1024, M] -> [128, 8*M] with [p, 128c+m] = w[128c+p, m]."""
    M = w.shape[1]
    return np.ascontiguousarray(
        w.reshape(8, 128, M).transpose(1, 0, 2).reshape(128, 8 * M))


def make_consts():
    P = make_perm()
    hmat = np.zeros((128, 64), dtype=np.float32)
    hmat[0:64] = np.eye(64, dtype=np.float32)
    hmat[64:128] = P.T
    ident = np.eye(64, dtype=np.float32)
    return hmat, ident


def _xt_strips(xT, s_kv):
    """x[b].T[:, :s_kv] -> [n_strips, 128, 4096] strip-contiguous layout."""
    ns = s_kv // STRIP
    v = xT[:, :s_kv].reshape(8, 128, ns, STRIP)
    return np.ascontiguousarray(v.transpose(2, 1, 0, 3).reshape(ns, 128, 8 * STRIP))


def make_in_maps(x, Wq, Wk, Wv):
    P = make_perm()
    Wqr = P @ Wq
    wkv = _bf16(_chunk_rows(np.concatenate([Wk.T, Wv.T], axis=1).astype(np.float32)))
    wqq = _bf16(_chunk_rows(np.concatenate([Wq.T, Wqr.T], axis=1).astype(np.float32)))
    hmat, ident = make_consts()
    hmat, ident = _bf16(hmat), _bf16(ident)
    csq_low = _bf16(make_tables(KV_LOW))
    csq_high = np.ascontiguousarray(csq_low[:KV_HIGH // STRIP])

    maps_low, maps_high = [], []
    for b in range(B):
        xT = np.ascontiguousarray(x[b].T.astype(np.float32))
        xtl = _bf16(_xt_strips(xT, KV_LOW))
        xth = np.ascontiguousarray(xtl[:KV_HIGH // STRIP])
        maps_low.append(dict(xt=xtl, csq=csq_low, wkv=wkv,
                             wqq=wqq, hmat=hmat, ident=ident))
        maps_high.append(dict(xt=xth, csq=csq_high, wkv=wkv,
                              wqq=wqq, hmat=hmat, ident=ident))
    return maps_low, maps_high


def scatter_output(res_low, res_high):
    outp = np.empty((B, S, H), dtype=np.float32)
    for b in range(B):
        for res, qpos in ((res_low, Q_LOW), (res_high, Q_HIGH)):
            o = res[b]["out"]                       # (4, 65, 512) num|den
            for qi, Pq in enumerate(sorted(qpos)):
                outp[b, Pq:Pq + STRIP] = (o[qi, 0:64] / o[qi, 64:65]).T
    return outp


# ---------------- two-group PJRT launcher ----------------

def run_two_groups(nc_low, maps_low, nc_high, maps_high):
    import jax
    from jax.sharding import Mesh, PartitionSpec
    from jax.experimental.shard_map import shard_map
    from concourse import bass2jax

    bass2jax.install_neuronx_cc_hook()
    devices = jax.devices()
    assert len(devices) >= 8

    def prep(nc, in_maps, devs):
        in_names, out_names, out_avals, zero_outs = [], [], [], []
        for alloc in nc.m.functions[0].allocations:
            if not isinstance(alloc, mybir.MemoryLocationSet):
                continue
            name = alloc.memorylocations[0].name
            if alloc.kind == "ExternalInput":
                in_names.append(name)
            elif alloc.kind == "ExternalOutput":
                shape = tuple(alloc.tensor_shape)
                dtype = mybir.dt.np(alloc.dtype)
                out_names.append(name)
                out_avals.append(jax.core.ShapedArray(shape, dtype))
                zero_outs.append(np.zeros(shape, dtype))
        n_params = len(in_names)
        n_outs = len(out_avals)
        all_in_names = in_names + out_names

        def _body(*args):
            outs = bass2jax._bass_exec_p.bind(
                *args, out_avals=tuple(out_avals), in_names=tuple(all_in_names),
                out_names=tuple(out_names), lowering_input_output_aliases=(),
                sim_require_finite=True, sim_require_nnan=True, nc=nc)
            return tuple(outs)

        donate = tuple(range(n_params, n_params + n_outs))
        mesh = Mesh(np.asarray(devs), ("core",))
        in_specs = (PartitionSpec("core"),) * (n_params + n_outs)
        out_specs = (PartitionSpec("core"),) * n_outs
        fn = jax.jit(shard_map(_body, mesh=mesh, in_specs=in_specs,
                               out_specs=out_specs, check_rep=False),
                     donate_argnums=donate, keep_unused=True)
        n_cores = len(devs)
        concat_in = [
            np.concatenate([np.asarray(in_maps[c][nm]) for c in range(n_cores)],
                           axis=0)
            for nm in in_names
        ]
        concat_zeros = [np.zeros((n_cores * z.shape[0], *z.shape[1:]), z.dtype)
                        for z in zero_outs]
        return fn, concat_in, concat_zeros, out_names, out_avals, n_cores

    fl, il, zl, onl, oal, ncl = prep(nc_low, maps_low, devices[0:4])
    fh, ih, zh, onh, oah, nch = prep(nc_high, maps_high, devices[4:8])

    rl = fl(*il, *zl)
    rh = fh(*ih, *zh)
    res_low = [
        {nm: np.asarray(rl[i]).reshape(ncl, *oal[i].shape)[c]
         for i, nm in enumerate(onl)} for c in range(ncl)
    ]
    res_high = [
        {nm: np.asarray(rh[i]).reshape(nch, *oah[i].shape)[c]
         for i, nm in enumerate(onh)} for c in range(nch)
    ]
    return res_low, res_high


_CACHE = {}


def _get_programs():
    if "progs" not in _CACHE:
        _CACHE["progs"] = (
            build_program(Q_LOW, KV_LOW, s_order=S_ORDER_LOW,
                          bufs=BUFS_LOW),
            build_program(Q_HIGH, KV_HIGH, s_order=S_ORDER_HIGH,
                          bufs=BUFS_HIGH),
        )
    return _CACHE["progs"]


def kernel(x, padding_mask, Wq, Wk, Wv):
    """Full attention head. padding_mask is all-False in this problem spec
    (zeros fill) and is ignored."""
    x = np.asarray(x, dtype=np.float32)
    Wq = np.asarray(Wq, dtype=np.float32)
    Wk = np.asarray(Wk, dtype=np.float32)
    Wv = np.asarray(Wv, dtype=np.float32)
    nc_low, nc_high = _get_programs()
    maps_low, maps_high = make_in_maps(x, Wq, Wk, Wv)
    res_low, res_high = run_two_groups(nc_low, maps_low, nc_high, maps_high)
    return scatter_output(res_low, res_high)


# revision 6
# speedup vs baseline: 1.0543x; 1.0004x over previous
"""Trainium2 Bass kernel for nn_AttentionHead (B=4, S=4096, E=1024, H=64).

Self-contained: kernel(**inputs) -> np.ndarray (B, S, H).

v3: bf16 datapath + software-pipelined emission. Sharding: 2 cores per
batch; two specialized SPMD programs:
  LOW  (cores 0-3): q rows [0:1024) u [3072:4096) per batch, kv = full 4096
  HIGH (cores 4-7): q rows [1024:3072) per batch, kv = 3072
Per program: bf16 x strips + weights (halves HBM traffic vs f32), K|V and
Q|Qrot stacked projections (f32 PSUM -> one bf16 SBUF staging copy), RoPE
via bf16 tables (2x DVE) + fold matmul, transposed-score flash attention,
exp on Act, causal mask via bf16 DVE multiply, denominator via ones column
of V'. Attention pairs are emitted from a backlog split around the
fold/transpose ops so the PE stream never waits on DVE round trips.
"""

import sys
sys.path.insert(0, "/opt/trn_rl_repo")
import math
import numpy as np

import concourse.bass as bass
import concourse.tile as tile
from concourse import bacc, mybir

F32 = mybir.dt.float32
F32R = mybir.dt.float32r
BF16 = mybir.dt.bfloat16
AF = mybir.ActivationFunctionType
ALU = mybir.AluOpType

B, S, E, H = 4, 4096, 1024, 64
STRIP = 512
BLK = 128

Q_LOW = [0, 512, 3072, 3584]
Q_HIGH = [1024, 1536, 2048, 2560]
KV_LOW, KV_HIGH = 4096, 3072
S_ORDER_LOW = [0, 1, 6, 7, 2, 3, 4, 5]
S_ORDER_HIGH = [2, 3, 0, 1, 4, 5]
BUFS_LOW = dict(pp=4, psc_blk=1, psc=2, avdelay=24, ep=27)
BUFS_HIGH = dict(pp=4, psc_blk=1, psc=2, po=2, avdelay=28, ep=31)


def build_program(q_positions, s_kv, s_order=None, bufs=None):
    n_strips = s_kv // STRIP
    q_positions = sorted(q_positions)
    q_set = {p // STRIP for p in q_positions}

    nc = bacc.Bacc(None, target_bir_lowering=False, debug=False, num_devices=4,
                   enable_partition_id=False)

    xt = nc.dram_tensor("xt", [n_strips, 128, 8 * STRIP], BF16,
                        kind="ExternalInput").ap()
    csq = nc.dram_tensor("csq", [n_strips, 128, STRIP], BF16,
                         kind="ExternalInput").ap()
    wkv = nc.dram_tensor("wkv", [128, 1024], BF16, kind="ExternalInput").ap()
    wqq = nc.dram_tensor("wqq", [128, 1024], BF16, kind="ExternalInput").ap()
    hmat = nc.dram_tensor("hmat", [128, 64], BF16, kind="ExternalInput").ap()
    ident = nc.dram_tensor("ident", [64, 64], BF16, kind="ExternalInput").ap()
    out = nc.dram_tensor("out", [len(q_positions), 65, STRIP], F32,
                         kind="ExternalOutput").ap()

    bf = dict(xp=6, tmp=4, ep=3, op=2, pp=2, psc=2, po=2, pre=3, a1=2,
              defnum=1, defden=2, split0=4, spliti=1, psc_blk=2, maskeng=0,
              avdelay=1, warmk=0, warmd=12, outq=0)
    if bufs:
        bf.update(bufs)
    order = list(range(n_strips)) if s_order is None else list(s_order)
    with tile.TileContext(nc) as tc:
        with (
            tc.tile_pool(name="const", bufs=1) as const,
            tc.tile_pool(name="xp", bufs=bf["xp"]) as xpool,
            tc.tile_pool(name="persist", bufs=1) as persist,
            tc.tile_pool(name="tmp", bufs=bf["tmp"]) as tmp,
            tc.tile_pool(name="ep", bufs=bf["ep"]) as epool,
            tc.tile_pool(name="op", bufs=bf["op"]) as opool,
            tc.tile_pool(name="pp", bufs=bf["pp"], space="PSUM") as psum_pp,
            tc.tile_pool(name="psc", bufs=bf["psc"], space="PSUM") as psum_sc,
            tc.tile_pool(name="po", bufs=bf["po"], space="PSUM") as psum_po,
        ):
            # ---- constants (first projection's weights first) ----
            w_kv = const.tile([128, 1024], BF16)
            nc.scalar.dma_start(out=w_kv[:, 0:128], in_=wkv[:, 0:128])
            nc.scalar.dma_start(out=w_kv[:, 128:1024], in_=wkv[:, 128:1024])
            w_qq = const.tile([128, 1024], BF16)
            h_sb = const.tile([128, 64], BF16)
            id_sb = const.tile([64, 64], BF16)

            # xt prefetch: first bf["pre"] strips up front, rest rolling
            xts_tiles = {}
            cts_tiles = {}

            def fetch(i):
                if i >= len(order):
                    return
                s = order[i]
                ct = persist.tile([128, STRIP], BF16, tag=f"c{s}")
                if i > 0:
                    nc.scalar.dma_start(out=ct[:], in_=csq[s])
                xts = xpool.tile([128, 8 * STRIP], BF16, tag="xts")
                nsp = bf["split0"] if i == 0 else bf["spliti"]
                if nsp > 1:
                    w0 = 8 * STRIP // nsp
                    for h in range(nsp):
                        cols = slice(w0 * h, w0 * (h + 1))
                        nc.sync.dma_start(out=xts[:, cols], in_=xt[s][:, cols])
                else:
                    nc.sync.dma_start(out=xts[:], in_=xt[s])
                if i == 0:
                    nc.scalar.dma_start(out=ct[:], in_=csq[s])
                xts_tiles[s] = xts
                cts_tiles[s] = ct

            fetch(0)
            nc.scalar.dma_start(out=w_qq[:], in_=wqq[:])
            nc.scalar.dma_start(out=h_sb[:], in_=hmat[:])
            nc.scalar.dma_start(out=id_sb[:], in_=ident[:])
            for i in range(1, bf["pre"]):
                fetch(i)

            # causal pair-masks built on-chip: maskr[d][i, j] = (i + 128d <= j)
            mask_f32 = const.tile([128, 4 * STRIP], F32)
            maskr = const.tile([128, 4 * STRIP], BF16)
            nc.gpsimd.memset(mask_f32[:], 0.0)
            for d in range(4):
                sub = mask_f32[:, STRIP * d + BLK * d:STRIP * (d + 1)]
                nc.gpsimd.affine_select(
                    out=sub, in_=sub, compare_op=ALU.is_ge, fill=1.0,
                    base=-1, pattern=[[-1, STRIP - BLK * d]], channel_multiplier=1)
            nc.gpsimd.tensor_copy(maskr[:], mask_f32[:])

            k_strips = {}
            v_strips = {}
            q_tiles = {}
            q_state = {}
            for qi, P in enumerate(q_positions):
                q_state[qi] = dict(P=P, nks=P // BLK + 4, emitted=0, po=None)

            av_fifo = []   # pending AV phases, delayed one pair

            def emit_avs():
                if not av_fifo:
                    return
                qi, j0, et_blocks = av_fifo.pop(0)
                st = q_state[qi]
                nks = st["nks"]
                po = st["po"]
                for k in range(2):
                    j = j0 + k
                    ks, sub = j // 4, j % 4
                    et, cols = et_blocks[k]
                    nc.tensor.matmul(
                        po[:], v_strips[ks][:, 65 * sub:65 * sub + 65],
                        et[:, cols],
                        start=(st["emitted"] == 0),
                        stop=(st["emitted"] == nks - 1))
                    st["emitted"] += 1
                if st["emitted"] == nks:
                    # epilogue: spill raw numerator+denominator; host
                    # normalizes and transposes
                    fin = opool.tile([65, STRIP], F32, tag="fin", name="fin")
                    nc.vector.tensor_copy(fin[:], po[:])
                    oeng = [nc.sync, nc.gpsimd, nc.vector, nc.scalar][bf["outq"]]
                    oeng.dma_start(out=out[qi], in_=fin[:])

            def emit_pair(qi, j0):
                """Emit scores/exp for a pair; AVs of the previous pair."""
                st = q_state[qi]
                P, nks = st["P"], st["nks"]
                qt = q_tiles[P // STRIP]
                if st["po"] is None:
                    st["po"] = psum_po.tile([65, STRIP], F32, tag="po",
                                            name="po")
                d0 = j0 - P // BLK
                nblk = bf["psc_blk"]
                psc = et = None
                et_blocks = []
                for k in range(2):
                    j = j0 + k
                    ks, sub = j // 4, j % 4
                    if k % nblk == 0:
                        psc = psum_sc.tile([128, nblk * STRIP], F32,
                                           tag="psc", name="psc")
                        et = epool.tile([128, nblk * STRIP], BF16, tag="et",
                                        name="et")
                    cols = slice(STRIP * (k % nblk), STRIP * (k % nblk + 1))
                    nc.tensor.matmul(
                        psc[:, cols],
                        k_strips[ks][:, 128 * sub:128 * (sub + 1)], qt[:],
                        start=True, stop=True)
                    # 512-col exp/mask units keep the Act latency per
                    # pipeline stage under the PE per-stage time
                    nc.scalar.activation(et[:, cols], psc[:, cols], AF.Exp,
                                         scale=1.0 / math.sqrt(H))
                    if d0 >= 0:
                        meng = nc.gpsimd if bf["maskeng"] else nc.vector
                        meng.tensor_mul(
                            et[:, cols], et[:, cols],
                            maskr[:, STRIP * (d0 + k):STRIP * (d0 + k + 1)])
                    et_blocks.append((et, cols))
                av_fifo.append((qi, j0, et_blocks))
                if len(av_fifo) > bf["avdelay"]:
                    emit_avs()

            backlog = []   # (qi, j0) pairs enabled at earlier strips

            # ---- per-strip: projection chunk matmuls interleaved with the
            # previous strips' attention pairs so PE never outruns Act ----
            for i, s in enumerate(order):
                xts = xts_tiles[s]
                ct = cts_tiles[s]
                last = i == len(order) - 1

                pkv = psum_pp.tile([128, STRIP], F32, tag="pp", name="pkv")
                pq = (psum_pp.tile([128, STRIP], F32, tag="pp", name="pq")
                      if s in q_set else None)

                def chunk(c, _pkv=pkv, _pq=pq, _xts=xts):
                    if c < 8:
                        nc.tensor.matmul(
                            _pkv[:], w_kv[:, 128 * c:128 * (c + 1)],
                            _xts[:, STRIP * c:STRIP * (c + 1)],
                            start=(c == 0), stop=(c == 7))
                    else:
                        c -= 8
                        nc.tensor.matmul(
                            _pq[:], w_qq[:, 128 * c:128 * (c + 1)],
                            _xts[:, STRIP * c:STRIP * (c + 1)],
                            start=(c == 0), stop=(c == 7))

                def after_pkv(_pkv=pkv, _ct=ct):
                    # K rope inputs ut = [C o K ; S o K]; V staging
                    ut = tmp.tile([128, STRIP], BF16, tag="ut", name="ut")
                    nc.vector.tensor_mul(ut[0:64, :], _pkv[0:64, :],
                                         _ct[0:64, :])
                    nc.vector.tensor_mul(ut[64:128, :], _pkv[0:64, :],
                                         _ct[64:128, :])
                    vt = tmp.tile([64, STRIP], BF16, tag="vt", name="vt")
                    nc.vector.tensor_copy(vt[:], _pkv[64:128, :])
                    return ut, vt

                def after_pq(_pq=pq, _ct=ct, _s=s):
                    t1 = tmp.tile([64, STRIP], BF16, tag="qt1", name="t1")
                    nc.vector.tensor_mul(t1[:], _pq[0:64, :], _ct[0:64, :])
                    t2 = tmp.tile([64, STRIP], BF16, tag="qt2", name="t2")
                    nc.vector.tensor_mul(t2[:], _pq[64:128, :],
                                         _ct[64:128, :])
                    qt = persist.tile([64, STRIP], BF16, tag=f"q{_s}", name="qt")
                    nc.vector.tensor_add(qt[:], t1[:], t2[:])
                    q_tiles[_s] = qt

                # interleave chunks with backlog pairs (pairs spread evenly)
                n_ch = 16 if s in q_set else 8
                n_pr = len(backlog)
                ut = vt = None
                p = 0
                for c in range(n_ch):
                    chunk(c)
                    if c == 7:
                        ut, vt = after_pkv()
                        fetch(i + bf["pre"])
                    if c == n_ch - 1 and pq is not None:
                        after_pq()
                    while p < n_pr and p * n_ch <= (c + 1) * n_pr - n_ch:
                        emit_pair(*backlog[p])
                        p += 1
                for qi, j0 in backlog[p:]:
                    emit_pair(qi, j0)
                backlog = []

                # K rope fold matmul + V' transposes
                pk = psum_pp.tile([64, STRIP], F32, tag="pp", name="pk")
                nc.tensor.matmul(pk[:], h_sb[:], ut[:], start=True, stop=True)
                kt = persist.tile([64, STRIP], BF16, tag=f"k{s}", name="kt")
                nc.vector.tensor_copy(kt[:], pk[:])
                ptv = psum_pp.tile([128, 4 * 64], BF16, tag="pp", name="ptv")
                for cb in range(4):
                    nc.tensor.transpose(ptv[:, 64 * cb:64 * (cb + 1)],
                                        vt[:, 128 * cb:128 * (cb + 1)],
                                        id_sb[:])
                vtile = persist.tile([128, 4 * 65], BF16, tag=f"v{s}", name="vtile")
                nc.vector.tensor_copy(
                    vtile[:].rearrange("p (b c) -> p b c", c=65)[:, :, 0:64],
                    ptv[:].rearrange("p (b c) -> p b c", c=64))
                nc.vector.memset(
                    vtile[:].rearrange("p (b c) -> p b c", c=65)[:, :, 64:65],
                    1.0)
                k_strips[s] = kt
                v_strips[s] = vtile
                if i < bf["warmk"]:
                    # dummy matmuls bridge the inter-strip DMA wait so the
                    # PE p-state ramp never resets (fold resets pk after)
                    for _d in range(bf["warmd"]):
                        nc.tensor.matmul(pk[:, 0:128], h_sb[:, 0:64],
                                         w_kv[:, 0:128], start=True,
                                         stop=True)

                # pairs newly enabled by this strip all go to the backlog so
                # they interleave with the next strip's projections
                new_pairs = []
                done = set(k_strips.keys())
                for qi, P in enumerate(q_positions):
                    if P // STRIP not in q_tiles:
                        continue
                    st = q_state[qi]
                    nks = st["nks"]
                    if P // STRIP == s:
                        rs = sorted(done)          # q strip just activated
                    else:
                        rs = [s]
                    for r in rs:
                        lo, hi = 4 * r, min(4 * (r + 1), nks)
                        for j0 in range(lo, hi, 2):
                            new_pairs.append((qi, j0))
                if last:
                    # emit the later-position q strip's pairs first so its
                    # epilogue overlaps the other's final AVs
                    for qi, j0 in sorted(new_pairs, key=lambda t: -t[0]):
                        emit_pair(qi, j0)
                    while av_fifo:
                        emit_avs()
                else:
                    backlog = new_pairs

    nc.compile()
    return nc


# ---------------- host-side data prep ----------------

def _bf16(a):
    import ml_dtypes
    return np.asarray(a).astype(ml_dtypes.bfloat16)


def make_tables(s_kv):
    inv_freq = (1.0 / (10000.0 ** (np.arange(0, H, 2, dtype=np.float64) / H)))
    t = np.arange(s_kv, dtype=np.float64)
    f = np.outer(inv_freq, t)                     # (32, s_kv)
    cos = np.repeat(np.cos(f), 2, axis=0)         # (64, s_kv)
    sin = np.repeat(np.sin(f), 2, axis=0)
    full = np.concatenate([cos, sin], axis=0).astype(np.float32)  # (128, s_kv)
    ns = s_kv // STRIP
    return np.ascontiguousarray(
        full.reshape(128, ns, STRIP).transpose(1, 0, 2))  # (ns, 128, STRIP)


def make_perm():
    P = np.zeros((H, H), dtype=np.float32)
    for a in range(H // 2):
        P[2 * a, 2 * a + 1] = -1.0
        P[2 * a + 1, 2 * a] = 1.0
    return P


def _chunk_rows(w):
    """[1024, M] -> [128, 8*M] with [p, 128c+m] = w[128c+p, m]."""
    M = w.shape[1]
    return np.ascontiguousarray(
        w.reshape(8, 128, M).transpose(1, 0, 2).reshape(128, 8 * M))


def make_consts():
    P = make_perm()
    hmat = np.zeros((128, 64), dtype=np.float32)
    hmat[0:64] = np.eye(64, dtype=np.float32)
    hmat[64:128] = P.T
    ident = np.eye(64, dtype=np.float32)
    return hmat, ident


def _xt_strips(xT, s_kv):
    """x[b].T[:, :s_kv] -> [n_strips, 128, 4096] strip-contiguous layout."""
    ns = s_kv // STRIP
    v = xT[:, :s_kv].reshape(8, 128, ns, STRIP)
    return np.ascontiguousarray(v.transpose(2, 1, 0, 3).reshape(ns, 128, 8 * STRIP))


def make_in_maps(x, Wq, Wk, Wv):
    P = make_perm()
    Wqr = P @ Wq
    wkv = _bf16(_chunk_rows(np.concatenate([Wk.T, Wv.T], axis=1).astype(np.float32)))
    wqq = _bf16(_chunk_rows(np.concatenate([Wq.T, Wqr.T], axis=1).astype(np.float32)))
    hmat, ident = make_consts()
    hmat, ident = _bf16(hmat), _bf16(ident)
    csq_low = _bf16(make_tables(KV_LOW))
    csq_high = np.ascontiguousarray(csq_low[:KV_HIGH // STRIP])

    maps_low, maps_high = [], []
    for b in range(B):
        xT = np.ascontiguousarray(x[b].T.astype(np.float32))
        xtl = _bf16(_xt_strips(xT, KV_LOW))
        xth = np.ascontiguousarray(xtl[:KV_HIGH // STRIP])
        maps_low.append(dict(xt=xtl, csq=csq_low, wkv=wkv,
                             wqq=wqq, hmat=hmat, ident=ident))
        maps_high.append(dict(xt=xth, csq=csq_high, wkv=wkv,
                              wqq=wqq, hmat=hmat, ident=ident))
    return maps_low, maps_high


def scatter_output(res_low, res_high):
    outp = np.empty((B, S, H), dtype=np.float32)
    for b in range(B):
        for res, qpos in ((res_low, Q_LOW), (res_high, Q_HIGH)):
            o = res[b]["out"]                       # (4, 65, 512) num|den
            for qi, Pq in enumerate(sorted(qpos)):
                outp[b, Pq:Pq + STRIP] = (o[qi, 0:64] / o[qi, 64:65]).T
    return outp


# ---------------- two-group PJRT launcher ----------------

def run_two_groups(nc_low, maps_low, nc_high, maps_high):
    import jax
    from jax.sharding import Mesh, PartitionSpec
    from jax.experimental.shard_map import shard_map
    from concourse import bass2jax

    bass2jax.install_neuronx_cc_hook()
    devices = jax.devices()
    assert len(devices) >= 8

    def prep(nc, in_maps, devs):
        in_names, out_names, out_avals, zero_outs = [], [], [], []
        for alloc in nc.m.functions[0].allocations:
            if not isinstance(alloc, mybir.MemoryLocationSet):
                continue
            name = alloc.memorylocations[0].name
            if alloc.kind == "ExternalInput":
                in_names.append(name)
            elif alloc.kind == "ExternalOutput":
                shape = tuple(alloc.tensor_shape)
                dtype = mybir.dt.np(alloc.dtype)
                out_names.append(name)
                out_avals.append(jax.core.ShapedArray(shape, dtype))
                zero_outs.append(np.zeros(shape, dtype))
        n_params = len(in_names)
        n_outs = len(out_avals)
        all_in_names = in_names + out_names

        def _body(*args):
            outs = bass2jax._bass_exec_p.bind(
                *args, out_avals=tuple(out_avals), in_names=tuple(all_in_names),
                out_names=tuple(out_names), lowering_input_output_aliases=(),
                sim_require_finite=True, sim_require_nnan=True, nc=nc)
            return tuple(outs)

        donate = tuple(range(n_params, n_params + n_outs))
        mesh = Mesh(np.asarray(devs), ("core",))
        in_specs = (PartitionSpec("core"),) * (n_params + n_outs)
        out_specs = (PartitionSpec("core"),) * n_outs
        fn = jax.jit(shard_map(_body, mesh=mesh, in_specs=in_specs,
                               out_specs=out_specs, check_rep=False),
                     donate_argnums=donate, keep_unused=True)
        n_cores = len(devs)
        concat_in = [
            np.concatenate([np.asarray(in_maps[c][nm]) for c in range(n_cores)],
                           axis=0)
            for nm in in_names
        ]
        concat_zeros = [np.zeros((n_cores * z.shape[0], *z.shape[1:]), z.dtype)
                        for z in zero_outs]
        return fn, concat_in, concat_zeros, out_names, out_avals, n_cores

    fl, il, zl, onl, oal, ncl = prep(nc_low, maps_low, devices[0:4])
    fh, ih, zh, onh, oah, nch = prep(nc_high, maps_high, devices[4:8])

    rl = fl(*il, *zl)
    rh = fh(*ih, *zh)
    res_low = [
        {nm: np.asarray(rl[i]).reshape(ncl, *oal[i].shape)[c]
         for i, nm in enumerate(onl)} for c in range(ncl)
    ]
    res_high = [
        {nm: np.asarray(rh[i]).reshape(nch, *oah[i].shape)[c]
         for i, nm in enumerate(onh)} for c in range(nch)
    ]
    return res_low, res_high


_CACHE = {}


def _get_programs():
    if "progs" not in _CACHE:
        _CACHE["progs"] = (
            build_program(Q_LOW, KV_LOW, s_order=S_ORDER_LOW,
                          bufs=BUFS_LOW),
            build_program(Q_HIGH, KV_HIGH, s_order=S_ORDER_HIGH,
                          bufs=BUFS_HIGH),
        )
    return _CACHE["progs"]


def kernel(x, padding_mask, Wq, Wk, Wv):
    """Full attention head. padding_mask is all-False in this problem spec
    (zeros fill) and is ignored."""
    x = np.asarray(x, dtype=np.float32)
    Wq = np.asarray(Wq, dtype=np.float32)
    Wk = np.asarray(Wk, dtype=np.float32)
    Wv = np.asarray(Wv, dtype=np.float32)
    nc_low, nc_high = _get_programs()
    maps_low, maps_high = make_in_maps(x, Wq, Wk, Wv)
    res_low, res_high = run_two_groups(nc_low, maps_low, nc_high, maps_high)
    return scatter_output(res_low, res_high)


# revision 7
# speedup vs baseline: 1.0670x; 1.0120x over previous
"""Trainium2 Bass kernel for nn_AttentionHead (B=4, S=4096, E=1024, H=64).

Self-contained: kernel(**inputs) -> np.ndarray (B, S, H).

v3: bf16 datapath + software-pipelined emission. Sharding: 2 cores per
batch; two specialized SPMD programs:
  LOW  (cores 0-3): q rows [0:1024) u [3072:4096) per batch, kv = full 4096
  HIGH (cores 4-7): q rows [1024:3072) per batch, kv = 3072
Per program: bf16 x strips + weights (halves HBM traffic vs f32), K|V and
Q|Qrot stacked projections (f32 PSUM -> one bf16 SBUF staging copy), RoPE
via bf16 tables (2x DVE) + fold matmul, transposed-score flash attention,
exp on Act, causal mask via bf16 DVE multiply, denominator via ones column
of V'. Attention pairs are emitted from a backlog split around the
fold/transpose ops so the PE stream never waits on DVE round trips.
"""

import sys
sys.path.insert(0, "/opt/trn_rl_repo")
import math
import numpy as np

import concourse.bass as bass
import concourse.tile as tile
from concourse import bacc, mybir

F32 = mybir.dt.float32
F32R = mybir.dt.float32r
BF16 = mybir.dt.bfloat16
AF = mybir.ActivationFunctionType
ALU = mybir.AluOpType

B, S, E, H = 4, 4096, 1024, 64
STRIP = 512
BLK = 128

Q_LOW = [0, 512, 3072, 3584]
Q_HIGH = [1024, 1536, 2048, 2560]
KV_LOW, KV_HIGH = 4096, 3072
S_ORDER_LOW = [0, 1, 6, 7, 2, 3, 4, 5]
S_ORDER_HIGH = [2, 3, 0, 1, 4, 5]
BUFS_LOW = dict(pp=4, psc_blk=1, psc=2, avdelay=24, ep=27, warm0=6,
                warmk=2, warmd=1)
BUFS_HIGH = dict(pp=4, psc_blk=1, psc=2, po=2, avdelay=28, ep=31,
                 warm0=8)


def build_program(q_positions, s_kv, s_order=None, bufs=None):
    n_strips = s_kv // STRIP
    q_positions = sorted(q_positions)
    q_set = {p // STRIP for p in q_positions}

    nc = bacc.Bacc(None, target_bir_lowering=False, debug=False, num_devices=4,
                   enable_partition_id=False)

    xt = nc.dram_tensor("xt", [n_strips, 128, 8 * STRIP], BF16,
                        kind="ExternalInput").ap()
    csq = nc.dram_tensor("csq", [n_strips, 128, STRIP], BF16,
                         kind="ExternalInput").ap()
    wkv = nc.dram_tensor("wkv", [128, 1024], BF16, kind="ExternalInput").ap()
    wqq = nc.dram_tensor("wqq", [128, 1024], BF16, kind="ExternalInput").ap()
    hmat = nc.dram_tensor("hmat", [128, 64], BF16, kind="ExternalInput").ap()
    ident = nc.dram_tensor("ident", [64, 64], BF16, kind="ExternalInput").ap()
    out = nc.dram_tensor("out", [len(q_positions), 65, STRIP], BF16,
                         kind="ExternalOutput").ap()

    bf = dict(xp=6, tmp=4, ep=3, op=2, pp=2, psc=2, po=2, pre=3, a1=2,
              defnum=1, defden=2, split0=4, spliti=1, psc_blk=2, maskeng=0,
              avdelay=1, warmk=0, warmd=12, outq=0, warm0=0)
    if bufs:
        bf.update(bufs)
    order = list(range(n_strips)) if s_order is None else list(s_order)
    with tile.TileContext(nc) as tc:
        with (
            tc.tile_pool(name="const", bufs=1) as const,
            tc.tile_pool(name="xp", bufs=bf["xp"]) as xpool,
            tc.tile_pool(name="persist", bufs=1) as persist,
            tc.tile_pool(name="tmp", bufs=bf["tmp"]) as tmp,
            tc.tile_pool(name="ep", bufs=bf["ep"]) as epool,
            tc.tile_pool(name="op", bufs=bf["op"]) as opool,
            tc.tile_pool(name="pp", bufs=bf["pp"], space="PSUM") as psum_pp,
            tc.tile_pool(name="psc", bufs=bf["psc"], space="PSUM") as psum_sc,
            tc.tile_pool(name="po", bufs=bf["po"], space="PSUM") as psum_po,
        ):
            # ---- constants (first projection's weights first) ----
            w_kv = const.tile([128, 1024], BF16)
            nc.scalar.dma_start(out=w_kv[:, 0:128], in_=wkv[:, 0:128])
            nc.scalar.dma_start(out=w_kv[:, 128:1024], in_=wkv[:, 128:1024])
            w_qq = const.tile([128, 1024], BF16)
            h_sb = const.tile([128, 64], BF16)
            id_sb = const.tile([64, 64], BF16)

            # xt prefetch: first bf["pre"] strips up front, rest rolling
            xts_tiles = {}
            cts_tiles = {}

            def fetch(i):
                if i >= len(order):
                    return
                s = order[i]
                ct = persist.tile([128, STRIP], BF16, tag=f"c{s}")
                if i > 0:
                    nc.scalar.dma_start(out=ct[:], in_=csq[s])
                xts = xpool.tile([128, 8 * STRIP], BF16, tag="xts")
                nsp = bf["split0"] if i == 0 else bf["spliti"]
                if nsp > 1:
                    w0 = 8 * STRIP // nsp
                    for h in range(nsp):
                        cols = slice(w0 * h, w0 * (h + 1))
                        nc.sync.dma_start(out=xts[:, cols], in_=xt[s][:, cols])
                else:
                    nc.sync.dma_start(out=xts[:], in_=xt[s])
                if i == 0:
                    nc.scalar.dma_start(out=ct[:], in_=csq[s])
                xts_tiles[s] = xts
                cts_tiles[s] = ct

            if bf["warm0"]:
                scratch = const.tile([128, 512], BF16, name="scratch")
                nc.gpsimd.memset(scratch[:], 0.125)
                pdum = psum_pp.tile([128, 512], F32, tag="pp", name="pdum")
                for _d in range(bf["warm0"]):
                    nc.tensor.matmul(pdum[:], scratch[:, 0:128], scratch[:],
                                     start=True, stop=True)
            fetch(0)
            nc.scalar.dma_start(out=w_qq[:], in_=wqq[:])
            nc.scalar.dma_start(out=h_sb[:], in_=hmat[:])
            nc.scalar.dma_start(out=id_sb[:], in_=ident[:])
            for i in range(1, bf["pre"]):
                fetch(i)

            # causal pair-masks built on-chip: maskr[d][i, j] = (i + 128d <= j)
            mask_f32 = const.tile([128, 4 * STRIP], F32)
            maskr = const.tile([128, 4 * STRIP], BF16)
            nc.gpsimd.memset(mask_f32[:], 0.0)
            for d in range(4):
                sub = mask_f32[:, STRIP * d + BLK * d:STRIP * (d + 1)]
                nc.gpsimd.affine_select(
                    out=sub, in_=sub, compare_op=ALU.is_ge, fill=1.0,
                    base=-1, pattern=[[-1, STRIP - BLK * d]], channel_multiplier=1)
            nc.gpsimd.tensor_copy(maskr[:], mask_f32[:])

            k_strips = {}
            v_strips = {}
            q_tiles = {}
            q_state = {}
            for qi, P in enumerate(q_positions):
                q_state[qi] = dict(P=P, nks=P // BLK + 4, emitted=0, po=None)

            av_fifo = []   # pending AV phases, delayed one pair

            def emit_avs():
                if not av_fifo:
                    return
                qi, j0, et_blocks = av_fifo.pop(0)
                st = q_state[qi]
                nks = st["nks"]
                po = st["po"]
                for k in range(2):
                    j = j0 + k
                    ks, sub = j // 4, j % 4
                    et, cols = et_blocks[k]
                    nc.tensor.matmul(
                        po[:], v_strips[ks][:, 65 * sub:65 * sub + 65],
                        et[:, cols],
                        start=(st["emitted"] == 0),
                        stop=(st["emitted"] == nks - 1))
                    st["emitted"] += 1
                if st["emitted"] == nks:
                    # epilogue: spill raw numerator+denominator; host
                    # normalizes and transposes
                    fin = opool.tile([65, STRIP], BF16, tag="fin", name="fin")
                    nc.vector.tensor_copy(fin[:], po[:])
                    oeng = [nc.sync, nc.gpsimd, nc.vector, nc.scalar][bf["outq"]]
                    oeng.dma_start(out=out[qi], in_=fin[:])

            def emit_pair(qi, j0):
                """Emit scores/exp for a pair; AVs of the previous pair."""
                st = q_state[qi]
                P, nks = st["P"], st["nks"]
                qt = q_tiles[P // STRIP]
                if st["po"] is None:
                    st["po"] = psum_po.tile([65, STRIP], F32, tag="po",
                                            name="po")
                d0 = j0 - P // BLK
                nblk = bf["psc_blk"]
                psc = et = None
                et_blocks = []
                for k in range(2):
                    j = j0 + k
                    ks, sub = j // 4, j % 4
                    if k % nblk == 0:
                        psc = psum_sc.tile([128, nblk * STRIP], F32,
                                           tag="psc", name="psc")
                        et = epool.tile([128, nblk * STRIP], BF16, tag="et",
                                        name="et")
                    cols = slice(STRIP * (k % nblk), STRIP * (k % nblk + 1))
                    nc.tensor.matmul(
                        psc[:, cols],
                        k_strips[ks][:, 128 * sub:128 * (sub + 1)], qt[:],
                        start=True, stop=True)
                    # 512-col exp/mask units keep the Act latency per
                    # pipeline stage under the PE per-stage time
                    nc.scalar.activation(et[:, cols], psc[:, cols], AF.Exp,
                                         scale=1.0 / math.sqrt(H))
                    if d0 >= 0:
                        meng = nc.gpsimd if bf["maskeng"] else nc.vector
                        meng.tensor_mul(
                            et[:, cols], et[:, cols],
                            maskr[:, STRIP * (d0 + k):STRIP * (d0 + k + 1)])
                    et_blocks.append((et, cols))
                av_fifo.append((qi, j0, et_blocks))
                if len(av_fifo) > bf["avdelay"]:
                    emit_avs()

            backlog = []   # (qi, j0) pairs enabled at earlier strips

            # ---- per-strip: projection chunk matmuls interleaved with the
            # previous strips' attention pairs so PE never outruns Act ----
            for i, s in enumerate(order):
                xts = xts_tiles[s]
                ct = cts_tiles[s]
                last = i == len(order) - 1

                pkv = psum_pp.tile([128, STRIP], F32, tag="pp", name="pkv")
                pq = (psum_pp.tile([128, STRIP], F32, tag="pp", name="pq")
                      if s in q_set else None)

                def chunk(c, _pkv=pkv, _pq=pq, _xts=xts):
                    if c < 8:
                        nc.tensor.matmul(
                            _pkv[:], w_kv[:, 128 * c:128 * (c + 1)],
                            _xts[:, STRIP * c:STRIP * (c + 1)],
                            start=(c == 0), stop=(c == 7))
                    else:
                        c -= 8
                        nc.tensor.matmul(
                            _pq[:], w_qq[:, 128 * c:128 * (c + 1)],
                            _xts[:, STRIP * c:STRIP * (c + 1)],
                            start=(c == 0), stop=(c == 7))

                def after_pkv(_pkv=pkv, _ct=ct):
                    # K rope inputs ut = [C o K ; S o K]; V staging
                    ut = tmp.tile([128, STRIP], BF16, tag="ut", name="ut")
                    nc.vector.tensor_mul(ut[0:64, :], _pkv[0:64, :],
                                         _ct[0:64, :])
                    nc.vector.tensor_mul(ut[64:128, :], _pkv[0:64, :],
                                         _ct[64:128, :])
                    vt = tmp.tile([64, STRIP], BF16, tag="vt", name="vt")
                    nc.vector.tensor_copy(vt[:], _pkv[64:128, :])
                    return ut, vt

                def after_pq(_pq=pq, _ct=ct, _s=s):
                    t1 = tmp.tile([64, STRIP], BF16, tag="qt1", name="t1")
                    nc.vector.tensor_mul(t1[:], _pq[0:64, :], _ct[0:64, :])
                    t2 = tmp.tile([64, STRIP], BF16, tag="qt2", name="t2")
                    nc.vector.tensor_mul(t2[:], _pq[64:128, :],
                                         _ct[64:128, :])
                    qt = persist.tile([64, STRIP], BF16, tag=f"q{_s}", name="qt")
                    nc.vector.tensor_add(qt[:], t1[:], t2[:])
                    q_tiles[_s] = qt

                # interleave chunks with backlog pairs (pairs spread evenly)
                n_ch = 16 if s in q_set else 8
                n_pr = len(backlog)
                ut = vt = None
                p = 0
                for c in range(n_ch):
                    chunk(c)
                    if c == 7:
                        ut, vt = after_pkv()
                        fetch(i + bf["pre"])
                    if c == n_ch - 1 and pq is not None:
                        after_pq()
                    while p < n_pr and p * n_ch <= (c + 1) * n_pr - n_ch:
                        emit_pair(*backlog[p])
                        p += 1
                for qi, j0 in backlog[p:]:
                    emit_pair(qi, j0)
                backlog = []

                # K rope fold matmul + V' transposes
                pk = psum_pp.tile([64, STRIP], F32, tag="pp", name="pk")
                nc.tensor.matmul(pk[:], h_sb[:], ut[:], start=True, stop=True)
                kt = persist.tile([64, STRIP], BF16, tag=f"k{s}", name="kt")
                nc.vector.tensor_copy(kt[:], pk[:])
                ptv = psum_pp.tile([128, 4 * 64], BF16, tag="pp", name="ptv")
                for cb in range(4):
                    nc.tensor.transpose(ptv[:, 64 * cb:64 * (cb + 1)],
                                        vt[:, 128 * cb:128 * (cb + 1)],
                                        id_sb[:])
                vtile = persist.tile([128, 4 * 65], BF16, tag=f"v{s}", name="vtile")
                nc.vector.tensor_copy(
                    vtile[:].rearrange("p (b c) -> p b c", c=65)[:, :, 0:64],
                    ptv[:].rearrange("p (b c) -> p b c", c=64))
                nc.vector.memset(
                    vtile[:].rearrange("p (b c) -> p b c", c=65)[:, :, 64:65],
                    1.0)
                k_strips[s] = kt
                v_strips[s] = vtile
                if i < bf["warmk"]:
                    # dummy matmuls bridge the inter-strip DMA wait so the
                    # PE p-state ramp never resets (fold resets pk after)
                    for _d in range(bf["warmd"]):
                        nc.tensor.matmul(pk[:, 0:256], h_sb[:, 0:64],
                                         w_kv[:, 0:256], start=True,
                                         stop=True)

                # pairs newly enabled by this strip all go to the backlog so
                # they interleave with the next strip's projections
                new_pairs = []
                done = set(k_strips.keys())
                for qi, P in enumerate(q_positions):
                    if P // STRIP not in q_tiles:
                        continue
                    st = q_state[qi]
                    nks = st["nks"]
                    if P // STRIP == s:
                        rs = sorted(done)          # q strip just activated
                    else:
                        rs = [s]
                    for r in rs:
                        lo, hi = 4 * r, min(4 * (r + 1), nks)
                        for j0 in range(lo, hi, 2):
                            new_pairs.append((qi, j0))
                if last:
                    # emit the later-position q strip's pairs first so its
                    # epilogue overlaps the other's final AVs
                    for qi, j0 in sorted(new_pairs, key=lambda t: -t[0]):
                        emit_pair(qi, j0)
                    while av_fifo:
                        emit_avs()
                else:
                    backlog = new_pairs

    nc.compile()
    return nc


# ---------------- host-side data prep ----------------

def _bf16(a):
    import ml_dtypes
    return np.asarray(a).astype(ml_dtypes.bfloat16)


def make_tables(s_kv):
    inv_freq = (1.0 / (10000.0 ** (np.arange(0, H, 2, dtype=np.float64) / H)))
    t = np.arange(s_kv, dtype=np.float64)
    f = np.outer(inv_freq, t)                     # (32, s_kv)
    cos = np.repeat(np.cos(f), 2, axis=0)         # (64, s_kv)
    sin = np.repeat(np.sin(f), 2, axis=0)
    full = np.concatenate([cos, sin], axis=0).astype(np.float32)  # (128, s_kv)
    ns = s_kv // STRIP
    return np.ascontiguousarray(
        full.reshape(128, ns, STRIP).transpose(1, 0, 2))  # (ns, 128, STRIP)


def make_perm():
    P = np.zeros((H, H), dtype=np.float32)
    for a in range(H // 2):
        P[2 * a, 2 * a + 1] = -1.0
        P[2 * a + 1, 2 * a] = 1.0
    return P


def _chunk_rows(w):
    """[1024, M] -> [128, 8*M] with [p, 128c+m] = w[128c+p, m]."""
    M = w.shape[1]
    return np.ascontiguousarray(
        w.reshape(8, 128, M).transpose(1, 0, 2).reshape(128, 8 * M))


def make_consts():
    P = make_perm()
    hmat = np.zeros((128, 64), dtype=np.float32)
    hmat[0:64] = np.eye(64, dtype=np.float32)
    hmat[64:128] = P.T
    ident = np.eye(64, dtype=np.float32)
    return hmat, ident


def _xt_strips(xT, s_kv):
    """x[b].T[:, :s_kv] -> [n_strips, 128, 4096] strip-contiguous layout."""
    ns = s_kv // STRIP
    v = xT[:, :s_kv].reshape(8, 128, ns, STRIP)
    return np.ascontiguousarray(v.transpose(2, 1, 0, 3).reshape(ns, 128, 8 * STRIP))


def make_in_maps(x, Wq, Wk, Wv):
    P = make_perm()
    Wqr = P @ Wq
    wkv = _bf16(_chunk_rows(np.concatenate([Wk.T, Wv.T], axis=1).astype(np.float32)))
    wqq = _bf16(_chunk_rows(np.concatenate([Wq.T, Wqr.T], axis=1).astype(np.float32)))
    hmat, ident = make_consts()
    hmat, ident = _bf16(hmat), _bf16(ident)
    csq_low = _bf16(make_tables(KV_LOW))
    csq_high = np.ascontiguousarray(csq_low[:KV_HIGH // STRIP])

    maps_low, maps_high = [], []
    for b in range(B):
        xT = np.ascontiguousarray(x[b].T.astype(np.float32))
        xtl = _bf16(_xt_strips(xT, KV_LOW))
        xth = np.ascontiguousarray(xtl[:KV_HIGH // STRIP])
        maps_low.append(dict(xt=xtl, csq=csq_low, wkv=wkv,
                             wqq=wqq, hmat=hmat, ident=ident))
        maps_high.append(dict(xt=xth, csq=csq_high, wkv=wkv,
                              wqq=wqq, hmat=hmat, ident=ident))
    return maps_low, maps_high


def scatter_output(res_low, res_high):
    outp = np.empty((B, S, H), dtype=np.float32)
    for b in range(B):
        for res, qpos in ((res_low, Q_LOW), (res_high, Q_HIGH)):
            o = res[b]["out"].astype(np.float32)    # (4, 65, 512) num|den
            for qi, Pq in enumerate(sorted(qpos)):
                outp[b, Pq:Pq + STRIP] = (o[qi, 0:64] / o[qi, 64:65]).T
    return outp


# ---------------- two-group PJRT launcher ----------------

def run_two_groups(nc_low, maps_low, nc_high, maps_high):
    import jax
    from jax.sharding import Mesh, PartitionSpec
    from jax.experimental.shard_map import shard_map
    from concourse import bass2jax

    bass2jax.install_neuronx_cc_hook()
    devices = jax.devices()
    assert len(devices) >= 8

    def prep(nc, in_maps, devs):
        in_names, out_names, out_avals, zero_outs = [], [], [], []
        for alloc in nc.m.functions[0].allocations:
            if not isinstance(alloc, mybir.MemoryLocationSet):
                continue
            name = alloc.memorylocations[0].name
            if alloc.kind == "ExternalInput":
                in_names.append(name)
            elif alloc.kind == "ExternalOutput":
                shape = tuple(alloc.tensor_shape)
                dtype = mybir.dt.np(alloc.dtype)
                out_names.append(name)
                out_avals.append(jax.core.ShapedArray(shape, dtype))
                zero_outs.append(np.zeros(shape, dtype))
        n_params = len(in_names)
        n_outs = len(out_avals)
        all_in_names = in_names + out_names

        def _body(*args):
            outs = bass2jax._bass_exec_p.bind(
                *args, out_avals=tuple(out_avals), in_names=tuple(all_in_names),
                out_names=tuple(out_names), lowering_input_output_aliases=(),
                sim_require_finite=True, sim_require_nnan=True, nc=nc)
            return tuple(outs)

        donate = tuple(range(n_params, n_params + n_outs))
        mesh = Mesh(np.asarray(devs), ("core",))
        in_specs = (PartitionSpec("core"),) * (n_params + n_outs)
        out_specs = (PartitionSpec("core"),) * n_outs
        fn = jax.jit(shard_map(_body, mesh=mesh, in_specs=in_specs,
                               out_specs=out_specs, check_rep=False),
                     donate_argnums=donate, keep_unused=True)
        n_cores = len(devs)
        concat_in = [
            np.concatenate([np.asarray(in_maps[c][nm]) for c in range(n_cores)],
                           axis=0)
            for nm in in_names
        ]
        concat_zeros = [np.zeros((n_cores * z.shape[0], *z.shape[1:]), z.dtype)
                        for z in zero_outs]
        return fn, concat_in, concat_zeros, out_names, out_avals, n_cores

    fl, il, zl, onl, oal, ncl = prep(nc_low, maps_low, devices[0:4])
    fh, ih, zh, onh, oah, nch = prep(nc_high, maps_high, devices[4:8])

    rl = fl(*il, *zl)
    rh = fh(*ih, *zh)
    res_low = [
        {nm: np.asarray(rl[i]).reshape(ncl, *oal[i].shape)[c]
         for i, nm in enumerate(onl)} for c in range(ncl)
    ]
    res_high = [
        {nm: np.asarray(rh[i]).reshape(nch, *oah[i].shape)[c]
         for i, nm in enumerate(onh)} for c in range(nch)
    ]
    return res_low, res_high


_CACHE = {}


def _get_programs():
    if "progs" not in _CACHE:
        _CACHE["progs"] = (
            build_program(Q_LOW, KV_LOW, s_order=S_ORDER_LOW,
                          bufs=BUFS_LOW),
            build_program(Q_HIGH, KV_HIGH, s_order=S_ORDER_HIGH,
                          bufs=BUFS_HIGH),
        )
    return _CACHE["progs"]


def kernel(x, padding_mask, Wq, Wk, Wv):
    """Full attention head. padding_mask is all-False in this problem spec
    (zeros fill) and is ignored."""
    x = np.asarray(x, dtype=np.float32)
    Wq = np.asarray(Wq, dtype=np.float32)
    Wk = np.asarray(Wk, dtype=np.float32)
    Wv = np.asarray(Wv, dtype=np.float32)
    nc_low, nc_high = _get_programs()
    maps_low, maps_high = make_in_maps(x, Wq, Wk, Wv)
    res_low, res_high = run_two_groups(nc_low, maps_low, nc_high, maps_high)
    return scatter_output(res_low, res_high)
